# revision 1
# baseline (speedup 1.0000x reference)
"""Full-width attention (B=4, S=2048, D=1024, no head split) on 8 TRN2 cores.

Sharding: data-parallel over (batch, query-half) -> 8 shards. Core c handles
batch b = c//2, query rows [h*1024, (h+1)*1024) with h = c%2. Each core
computes K/V projections for its full batch (redundantly with its pair core),
Q projection for its query half, then scores^T -> exp -> AV locally.

Layout trick: everything is computed without any on-device transposes.
  - host passes x^T (d-major) per batch, plus W^T for each projection
  - Q^T[e,s] = (Wq^T)^T.T @ x^T   (lhsT=WqT, rhs=xT)  -> e on partitions
  - K^T[e,s] likewise, staged to DRAM scratch and re-streamed
  - V[s,e]   = (x^T).T @ Wv^T     (lhsT=xT,  rhs=WvT) -> s on partitions
  - scores^T[k,q] = KT.T @ QT (contract e)            -> k on partitions
  - softmax without max-subtraction (|scores| <= ~25, exp is safe in fp32):
    E = exp(scores^T / 8); rowsum via matmul with ones-vector rhs;
    out[q,e] = E.T @ V (contract k), scaled by 1/rowsum per partition.
  - bv folded in at the end: softmax rows sum to 1, so out += bv.
All matmuls run as float32r (1-pass FP22) at full PE speed. V is staged to
DRAM during projections and preloaded back to SBUF per q-chunk during the
(PE-bound) scores phase, so the AV matmuls are fully SBUF-fed.
"""

import math
from contextlib import ExitStack

import numpy as np

P = 128
B, S, D = 4, 2048, 1024
SQ = 1024  # query rows per core
KO = D // P  # 8 chunks of contraction dim
N_CORES = 8


def build_bass():
    from concourse import bacc
    import concourse.mybir as mybir
    from concourse.tile import TileContext

    f32 = mybir.dt.float32
    f32r = mybir.dt.float32r
    AF = mybir.ActivationFunctionType

    nc = bacc.Bacc(
        "TRN2",
        target_bir_lowering=False,
        debug=False,
        enable_asserts=False,
        num_devices=N_CORES,
    )

    xT = nc.dram_tensor("xT", [D, S], f32r, kind="ExternalInput")
    xn = nc.dram_tensor("xn", [S, D], f32r, kind="ExternalInput")
    xTq = nc.dram_tensor("xTq", [D, SQ], f32r, kind="ExternalInput")
    mT = nc.dram_tensor("mT", [D, D], f32r, kind="ExternalInput")
    wvT = nc.dram_tensor("wvT", [D, D], f32r, kind="ExternalInput")
    wcol = nc.dram_tensor("wcol", [P, KO], f32r, kind="ExternalInput")
    bvb = nc.dram_tensor("bvb", [P, D], f32, kind="ExternalInput")
    ones = nc.dram_tensor("ones", [P, 512], f32r, kind="ExternalInput")
    out = nc.dram_tensor("out", [SQ, D], f32, kind="ExternalOutput")

    xT_r = xT[:, :].rearrange("(ko p) s -> p ko s", p=P)
    xTq_r = xTq[:, :].rearrange("(ko p) s -> p ko s", p=P)
    mT_r = mT[:, :].rearrange("(ko p) e -> p ko e", p=P)
    wvT_r = wvT[:, :].rearrange("(ko p) e -> p ko e", p=P)

    with TileContext(nc) as tc, ExitStack() as ctx:
        qt_pool = ctx.enter_context(tc.tile_pool(name="qtp", bufs=1))
        kt_pool = ctx.enter_context(tc.tile_pool(name="ktp", bufs=1))
        cpool = ctx.enter_context(tc.tile_pool(name="cp", bufs=1))
        psA_p = ctx.enter_context(tc.tile_pool(name="psA", bufs=3, space="PSUM"))
        psB_p = ctx.enter_context(tc.tile_pool(name="psB", bufs=2, space="PSUM"))
        psC_p = ctx.enter_context(tc.tile_pool(name="psC", bufs=2, space="PSUM"))
        psR_p = ctx.enter_context(tc.tile_pool(name="psR", bufs=1, space="PSUM"))
        dram_p = ctx.enter_context(tc.tile_pool(name="drp", bufs=1, space="DRAM"))

        xq = qt_pool.tile([P, KO, SQ], f32r)  # raw x^T (query half), resident
        kt = kt_pool.tile([P, KO, S], f32r)  # (M x^T) "modified K^T", resident
        t3_dram = dram_p.tile([1, S], f32, tag="t3d", name="t3_dram")

        ones_t = cpool.tile([P, 512], f32r)
        nc.gpsimd.dma_start(ones_t[:], ones[:, :])
        wcol_t = cpool.tile([P, KO], f32r)
        nc.gpsimd.dma_start(wcol_t[:], wcol[:, :])

        inv_sqrt_dk = 1.0 / math.sqrt(D // 16)  # d_key = 64

        # PE warm-up: junk matmuls on the ones tile keep the HAM activity
        # window busy while the first real operands stream in, so the first
        # real matmuls run at 2.4 GHz instead of 1.2 GHz.
        warm = psR_p.tile([1, 512], f32, tag="psR", name="warm")
        for _ in range(22):
            nc.tensor.matmul(warm[:], ones_t[:, 0:1], ones_t[:, :])

        # ---- Phase A: V (to DRAM) and K^T (resident) from one xt pass ----
        with (
            tc.tile_pool(name="xtp", bufs=2) as xt_pool,
            tc.tile_pool(name="wp", bufs=2) as w_pool,
        ):
            wk = [
                w_pool.tile([P, KO, 512], f32r, tag="w", name=f"wm{half}")
                for half in range(2)
            ]
            xtv0 = xt_pool.tile([P, KO, 512], f32r, tag="xt", name="xtv0")
            for ko in range(KO):
                nc.sync.dma_start(wk[0][:, ko, :], mT_r[:, ko, 0:512])
                nc.sync.dma_start(xtv0[:, ko, :], xT_r[:, ko, 0:512])
            for ko in range(KO):
                nc.sync.dma_start(wk[1][:, ko, :], mT_r[:, ko, 512:1024])
            for sc in range(4):
                if sc == 0:
                    xt_c = xtv0
                else:
                    xt_c = xt_pool.tile([P, KO, 512], f32r, tag="xt", name=f"xtv{sc}")
                    for ko in range(KO):
                        nc.sync.dma_start(
                            xt_c[:, ko, :], xT_r[:, ko, sc * 512 : (sc + 1) * 512]
                        )
                # (M x^T) columns for this x chunk -> resident SBUF
                for eo in range(KO):
                    pa = psA_p.tile([P, 512], f32, tag="psA", name="pak")
                    wkh = wk[eo // 4]
                    col = (eo % 4) * P
                    for ko in range(KO):
                        nc.tensor.matmul(
                            pa[:], wkh[:, ko, col : col + P], xt_c[:, ko, :],
                            start=(ko == 0), stop=(ko == KO - 1),
                        )
                    nc.scalar.copy(kt[:, eo, sc * 512 : (sc + 1) * 512], pa[:])
                # per-key score bias t3 = x . (Wk^T bq), pre-scaled by 1/8
                t3p = psR_p.tile([1, 512], f32, tag="psR", name="t3p")
                for ko in range(KO):
                    nc.tensor.matmul(
                        t3p[:], wcol_t[:, ko : ko + 1], xt_c[:, ko, :],
                        start=(ko == 0), stop=(ko == KO - 1),
                    )
                t3r = xt_pool.tile([1, 512], f32, tag="t3r", name="t3r")
                nc.scalar.activation(t3r[:], t3p[:], AF.Identity, scale=inv_sqrt_dk)
                nc.sync.dma_start(t3_dram[0:1, sc * 512 : (sc + 1) * 512], t3r[:])

            # raw query-half x^T is the scores rhs; plain load (no projection),
            # overlapped with the tail of the projection compute
            for ko in range(KO):
                nc.sync.dma_start(xq[:, ko, :], xTq_r[:, ko, :])

        # ---------------- Phase C: attention ----------------
        with (
            tc.tile_pool(name="ep", bufs=1) as e_pool,
            tc.tile_pool(name="vsp", bufs=1) as vs_pool,
            tc.tile_pool(name="osp", bufs=2) as out_pool,
            tc.tile_pool(name="xrp", bufs=4) as xr_pool,
            tc.tile_pool(name="msc", bufs=1) as msc_pool,
        ):
            t3_t = msc_pool.tile([P, S // P], f32, tag="t3t", name="t3_t")
            nc.sync.dma_start(
                t3_t[:, :], t3_dram[0, :].rearrange("(c p) -> p c", p=P)
            )
            bvb_t = msc_pool.tile([P, D], f32, tag="bvb", name="bvb_t")
            nc.gpsimd.dma_start(bvb_t[:], bvb[:, :])
            # Wv is applied AFTER the attention sum: out = (E^T x / rowsum) Wv^T
            wv_sb = [
                vs_pool.tile([P, KO, 512], f32r, tag=f"wv{h}", name=f"wv_sb{h}")
                for h in range(2)
            ]
            for h in range(2):
                for ko in range(KO):
                    nc.sync.dma_start(
                        wv_sb[h][:, ko, :], wvT_r[:, ko, h * 512 : (h + 1) * 512]
                    )
            pxt_sb = vs_pool.tile([P, KO, 512], f32r, tag="pxt", name="pxt_sb")
            for qc in range(2):
                E = e_pool.tile([P, S // P, 512], f32r, tag="E", name="E")
                q_sl = xq[:, :, qc * 512 : (qc + 1) * 512]
                pr = psR_p.tile([1, 512], f32, tag="psR", name="pr")
                racc = msc_pool.tile([P, 512], f32r, tag="racc", name="racc")
                for kc in range(4):
                    for ks in range(4):
                        idx = kc * 4 + ks
                        pa = psA_p.tile([P, 512], f32, tag="psA", name="pas")
                        for eo in range(KO):
                            nc.tensor.matmul(
                                pa[:],
                                kt[:, eo, idx * P : (idx + 1) * P],
                                q_sl[:, eo, :],
                                start=(eo == 0), stop=(eo == KO - 1),
                            )
                        nc.scalar.activation(
                            E[:, idx, :], pa[:], AF.Exp, scale=inv_sqrt_dk,
                            bias=t3_t[:, idx : idx + 1],
                        )
                        if idx == 0:
                            nc.vector.tensor_copy(racc[:], E[:, 0, :])
                        else:
                            nc.vector.tensor_add(racc[:], racc[:], E[:, idx, :])
                # partition-reduce the accumulated rowsum with one ones-matmul,
                # then [1,512] -> per-partition recips [128,4] via DRAM bounce
                nc.tensor.matmul(pr[:], ones_t[:, 0:1], racc[:])
                rsum_row = msc_pool.tile([1, 512], f32, tag="rsr", name="rsum_row")
                nc.scalar.copy(rsum_row[:], pr[:])
                rs_dram = dram_p.tile([1, 512], f32, tag="rsd", name="rs_dram")
                nc.sync.dma_start(rs_dram[:, :], rsum_row[:, :])
                rsum_t = msc_pool.tile([P, 4], f32, tag="rst", name="rsum_t")
                nc.sync.dma_start(
                    rsum_t[:, :],
                    rs_dram[0, :].rearrange("(qs p) -> p qs", p=P),
                )
                recip = msc_pool.tile([P, 4], f32, tag="recip", name="recip")
                nc.vector.reciprocal(recip[:], rsum_t[:])

                # PX^T[d, q] = sum_k x[k, d] E[k, q]: x rows streamed from
                # DRAM, all 8 d-chunks accumulated across 8 PSUM banks.
                # bank order: outMM consumes psB/psC first, so evac them first
                pxt_ps = [
                    psB_p.tile([P, 512], f32, tag="psB", name="px0"),
                    psC_p.tile([P, 512], f32, tag="psC", name="px1"),
                    psB_p.tile([P, 512], f32, tag="psB", name="px2"),
                    psC_p.tile([P, 512], f32, tag="psC", name="px3"),
                    psA_p.tile([P, 512], f32, tag="psA", name="px4"),
                    psA_p.tile([P, 512], f32, tag="psA", name="px5"),
                    psA_p.tile([P, 512], f32, tag="psA", name="px6"),
                    psR_p.tile([P, 512], f32, tag="psR", name="px7"),
                ]
                for ko in range(S // P):
                    xr = xr_pool.tile([P, D], f32r, tag="xr", name="xr")
                    nc.sync.dma_start(xr[:], xn[ko * P : (ko + 1) * P, :])
                    for dc in range(KO):
                        nc.tensor.matmul(
                            pxt_ps[dc][:],
                            xr[:, dc * P : (dc + 1) * P],
                            E[:, ko, :],
                            start=(ko == 0), stop=(ko == S // P - 1),
                        )
                for dc in range(KO):
                    nc.scalar.copy(pxt_sb[:, dc, :], pxt_ps[dc][:])
                # out[q, e] = PX^T.T @ Wv^T, scaled by 1/rowsum, + bv
                for qs in range(4):
                    pb = psB_p.tile([P, 512], f32, tag="psB", name="avb")
                    pc = psC_p.tile([P, 512], f32, tag="psC", name="avc")
                    for ko in range(KO):
                        lh = pxt_sb[:, ko, qs * P : (qs + 1) * P]
                        nc.tensor.matmul(
                            pb[:], lh, wv_sb[0][:, ko, :],
                            start=(ko == 0), stop=(ko == KO - 1),
                        )
                        nc.tensor.matmul(
                            pc[:], lh, wv_sb[1][:, ko, :],
                            start=(ko == 0), stop=(ko == KO - 1),
                        )
                    row0 = qc * 512 + qs * P
                    for half, ps in ((0, pb), (1, pc)):
                        o = out_pool.tile([P, 512], f32, tag="ost", name="ost")
                        nc.scalar.activation(
                            o[:], ps[:], AF.Identity, scale=recip[:, qs : qs + 1]
                        )
                        nc.vector.tensor_add(
                            o[:], o[:], bvb_t[:, half * 512 : (half + 1) * 512]
                        )
                        nc.sync.dma_start(
                            out[row0 : row0 + P, half * 512 : (half + 1) * 512], o[:]
                        )

    nc.finalize()
    return nc


def make_in_maps(x, Wq, bq, Wk, bk, Wv, bv):
    """Build the 8 per-core input maps from full inputs."""
    x = np.asarray(x, dtype=np.float32)
    # weight-only constant folding: scores = x (Wq^T Wk) x^T + per-row-const
    # terms (softmax-invariant, dropped) + per-key bias x.(Wk^T bq).
    # lhsT for the modified-K projection is M^T = (Wq^T Wk)^T = Wk^T Wq.
    mTh = np.ascontiguousarray(
        (np.asarray(Wk, np.float64).T @ np.asarray(Wq, np.float64)).astype(
            np.float32
        )
    )
    wvT = np.ascontiguousarray(np.asarray(Wv, np.float32).T)
    w3 = (np.asarray(Wk, np.float64).T @ np.asarray(bq, np.float64)).astype(
        np.float32
    )
    wcol_np = np.ascontiguousarray(w3.reshape(KO, P).T)
    bvb = np.ascontiguousarray(
        np.broadcast_to(np.asarray(bv, np.float32), (P, D))
    )
    ones_np = np.ones((P, 512), np.float32)
    xT_b = [np.ascontiguousarray(x[b].T) for b in range(B)]
    in_maps = []
    for c in range(N_CORES):
        b, h = c // 2, c % 2
        in_maps.append(
            {
                "xT": xT_b[b],
                "xn": np.ascontiguousarray(x[b]),
                "xTq": np.ascontiguousarray(x[b, h * SQ : (h + 1) * SQ].T),
                "mT": mTh,
                "wvT": wvT,
                "wcol": wcol_np,
                "bvb": bvb,
                "ones": ones_np,
            }
        )
    return in_maps


_NC_CACHE = None


def get_nc():
    global _NC_CACHE
    if _NC_CACHE is None:
        _NC_CACHE = build_bass()
    return _NC_CACHE


def kernel(x, Wq, bq, Wk, bk, Wv, bv, **run_kwargs):
    from concourse.bass_utils import run_bass_kernel_spmd

    nc = get_nc()
    in_maps = make_in_maps(x, Wq, bq, Wk, bk, Wv, bv)
    res = run_bass_kernel_spmd(
        nc, in_maps, core_ids=list(range(N_CORES)), **run_kwargs
    )
    out = np.empty((B, S, D), dtype=np.float32)
    for c in range(N_CORES):
        b, h = c // 2, c % 2
        out[b, h * SQ : (h + 1) * SQ, :] = res.results[c]["out"]
    if run_kwargs.get("trace"):
        kernel.last_results = res
    return out



# revision 6
# speedup vs baseline: 1.1542x; 1.1542x over previous
"""Full-width attention (B=4, S=2048, D=1024, no head split) on 8 TRN2 cores.

Sharding: data-parallel over (batch, query-half) -> 8 shards. Core c handles
batch b = c//2, query rows [h*1024, (h+1)*1024) with h = c%2.

Zero-redundancy decomposition (12.88 GFLOP/core, the 103 GFLOP/8 floor):
the weight fold Wq^T Wk is applied to the QUERY side, not the key side:
  qm   = x_own A          (A = Wq^T Wk)        2.15 GFLOP   [own 1024 q only]
  S^T  = x_full^T . qm^T  (contract raw e)     4.29 GFLOP   [keys need NO proj]
  E    = exp(S/8 + t3),   t3 = x.(Wk^T bq)     (host-computed, ACT bias)
  PX^T = x^T E            (contract k)         4.29 GFLOP
  out  = (PX/rowsum) Wv^T + bv                 2.15 GFLOP
Per-core inputs are key-permuted (own query half first) so the same SPMD
program can slice "own queries" at columns 0..1023; attention is invariant
to a consistent key permutation of (xT, xnb, t3).

Precision: scores path (A, xT, qm) in fp32r (1-pass FP22, full PE rate).
V path (xn, E, Px, Wv) in bf16 — same PE rate, half the DMA/SBUF, and the
whole value tensor stays SBUF-resident so the PX phase does zero DMA.

Warm-up junk matmuls run off a memset tile (no DMA dependency) so the HAM
clock-gate opens while the first operands stream in.
"""

import math
from contextlib import ExitStack

import numpy as np

P = 128
B, S, D = 4, 2048, 1024
SQ = 1024  # query rows per core
KO = D // P  # 8 chunks of the d/e contraction dims
KC = S // P  # 16 key chunks
N_CORES = 8


def build_bass():
    from concourse import bacc
    import concourse.mybir as mybir
    from concourse.tile import TileContext

    f32 = mybir.dt.float32
    f32r = mybir.dt.float32r
    bf16 = mybir.dt.bfloat16
    AF = mybir.ActivationFunctionType

    nc = bacc.Bacc(
        "TRN2",
        target_bir_lowering=False,
        debug=False,
        enable_asserts=False,
        num_devices=N_CORES,
    )

    xT = nc.dram_tensor("xT", [D, S], f32r, kind="ExternalInput")
    a = nc.dram_tensor("a", [KO * D, P], f32r, kind="ExternalInput")
    xnb = nc.dram_tensor("xnb", [S, D], bf16, kind="ExternalInput")
    wvT = nc.dram_tensor("wvT", [D, D], bf16, kind="ExternalInput")
    t3 = nc.dram_tensor("t3", [P, KC], f32, kind="ExternalInput")
    ones = nc.dram_tensor("ones", [P, 256], f32r, kind="ExternalInput")
    bvb = nc.dram_tensor("bvb", [P, D], f32, kind="ExternalInput")
    out = nc.dram_tensor("out", [SQ, D], f32, kind="ExternalOutput")

    xT_r = xT[:, :].rearrange("(ko p) s -> p ko s", p=P)
    xnb_r = xnb[:, :].rearrange("(ko p) d -> p ko d", p=P)
    wvT_r = wvT[:, :].rearrange("(ko p) e -> p ko e", p=P)

    inv_sqrt_dk = 1.0 / math.sqrt(D // 16)  # d_key = 64

    with TileContext(nc) as tc, ExitStack() as ctx:
        xt_pool = ctx.enter_context(tc.tile_pool(name="xtp", bufs=1))
        qm_pool = ctx.enter_context(tc.tile_pool(name="qmp", bufs=1))
        msc_pool = ctx.enter_context(tc.tile_pool(name="msc", bufs=1))
        psA_p = ctx.enter_context(tc.tile_pool(name="psA", bufs=3, space="PSUM"))
        psB_p = ctx.enter_context(tc.tile_pool(name="psB", bufs=2, space="PSUM"))
        psC_p = ctx.enter_context(tc.tile_pool(name="psC", bufs=2, space="PSUM"))
        psR_p = ctx.enter_context(tc.tile_pool(name="psR", bufs=1, space="PSUM"))
        dram_p = ctx.enter_context(tc.tile_pool(name="drp", bufs=1, space="DRAM"))

        xt = xt_pool.tile([P, KO, S], f32r)  # raw x^T, resident
        qmT = qm_pool.tile([P, KO, SQ], f32r)  # (x_own A)^T, resident

        warm = msc_pool.tile([P, 256], f32r, tag="warm", name="warm")
        nc.gpsimd.dma_start(warm[:], ones[:, :])
        t3_t = msc_pool.tile([P, KC], f32, tag="t3t", name="t3_t")
        nc.sync.dma_start(t3_t[:], t3[:, :])

        # PE warm-up off the memset tile: no DMA dependency, so the HAM
        # activity window opens while the first real operands stream in.
        warm_ps = psR_p.tile([1, 256], f32, tag="psR", name="warm_ps")
        for _ in range(20):
            nc.tensor.matmul(warm_ps[:], warm[:, 0:1], warm[:, :])

        # ---- Phase Q: qm^T[e, q] = A^T x_own^T (A resident, phase-scoped) ----
        with tc.tile_pool(name="ap", bufs=1) as a_pool:
            a_t = a_pool.tile([P, KO * KO, P], f32r)
            for eo in range(KO):
                # host pre-permuted A so each eo block is one contiguous chunk
                nc.sync.dma_start(
                    a_t[:, eo * KO : (eo + 1) * KO, :],
                    a[eo * D : (eo + 1) * D, :].rearrange("(ko p) e -> p ko e", p=P),
                )
            # xT arrives: own-qc0 columns, own-qc1 columns, then the rest
            for ko in range(KO):
                nc.gpsimd.dma_start(xt[:, ko, 0:512], xT_r[:, ko, 0:512])
            for ko in range(KO):
                nc.gpsimd.dma_start(xt[:, ko, 512:1024], xT_r[:, ko, 512:1024])
            for ko in range(KO):
                nc.gpsimd.dma_start(xt[:, ko, 1024:2048], xT_r[:, ko, 1024:2048])

            for qc in range(2):
                for eo in range(KO):
                    pa = psA_p.tile([P, 512], f32, tag="psA", name="paq")
                    for ko in range(KO):
                        nc.tensor.matmul(
                            pa[:],
                            a_t[:, eo * KO + ko, :],
                            xt[:, ko, qc * 512 : (qc + 1) * 512],
                            start=(ko == 0),
                            stop=(ko == KO - 1),
                        )
                    nc.scalar.copy(qmT[:, eo, qc * 512 : (qc + 1) * 512], pa[:])

        # ---------------- Phase C: attention ----------------
        with (
            tc.tile_pool(name="ep", bufs=1) as e_pool,
            tc.tile_pool(name="vsp", bufs=1) as vs_pool,
            tc.tile_pool(name="osp", bufs=2) as out_pool,
        ):
            xnb_t = vs_pool.tile([P, KC, D], bf16, tag="xnb", name="xnb_t")
            for ko in range(KC):
                nc.gpsimd.dma_start(xnb_t[:, ko, :], xnb_r[:, ko, :])
            wv_sb = [
                vs_pool.tile([P, KO, 512], bf16, tag=f"wv{h}", name=f"wv_sb{h}")
                for h in range(2)
            ]
            for h in range(2):
                for ko in range(KO):
                    nc.gpsimd.dma_start(
                        wv_sb[h][:, ko, :], wvT_r[:, ko, h * 512 : (h + 1) * 512]
                    )
            bvb_t = msc_pool.tile([P, D], f32, tag="bvb", name="bvb_t")
            nc.gpsimd.dma_start(bvb_t[:], bvb[:, :])
            pxt_sb = vs_pool.tile([P, KO, 512], bf16, tag="pxt", name="pxt_sb")

            for qc in range(2):
                E = e_pool.tile([P, KC, 512], bf16, tag="E", name="E")
                racc = msc_pool.tile([P, 512], f32r, tag="racc", name="racc")
                for kc in range(KC):
                    pa = psA_p.tile([P, 512], f32, tag="psA", name="pas")
                    for eo in range(KO):
                        nc.tensor.matmul(
                            pa[:],
                            xt[:, eo, kc * P : (kc + 1) * P],
                            qmT[:, eo, qc * 512 : (qc + 1) * 512],
                            start=(eo == 0),
                            stop=(eo == KO - 1),
                        )
                    nc.scalar.activation(
                        E[:, kc, :], pa[:], AF.Exp, scale=inv_sqrt_dk,
                        bias=t3_t[:, kc : kc + 1],
                    )
                    if kc == 0:
                        nc.vector.tensor_copy(racc[:], E[:, 0, :])
                    else:
                        nc.vector.tensor_add(racc[:], racc[:], E[:, kc, :])
                # partition-reduce rowsum with one ones-matmul, then
                # [1,512] -> per-partition recips [128,4] via DRAM bounce
                pr = psR_p.tile([1, 512], f32, tag="psR", name="pr")
                nc.tensor.matmul(pr[:], warm[:, 0:1], racc[:])
                rsum_row = msc_pool.tile([1, 512], f32, tag="rsr", name="rsum_row")
                nc.scalar.copy(rsum_row[:], pr[:])
                rs_dram = dram_p.tile([1, 512], f32, tag="rsd", name="rs_dram")
                nc.sync.dma_start(rs_dram[:, :], rsum_row[:, :])
                rsum_t = msc_pool.tile([P, 4], f32, tag="rst", name="rsum_t")
                nc.sync.dma_start(
                    rsum_t[:, :], rs_dram[0, :].rearrange("(qs p) -> p qs", p=P)
                )
                recip = msc_pool.tile([P, 4], f32, tag="recip", name="recip")
                nc.vector.reciprocal(recip[:], rsum_t[:])

                # PX^T[d, q] = sum_k x[k, d] E[k, q]: fully SBUF-fed (bf16).
                # bank order: outMM consumes psB/psC first, so evac them first
                pxt_ps = [
                    psB_p.tile([P, 512], f32, tag="psB", name="px0"),
                    psC_p.tile([P, 512], f32, tag="psC", name="px1"),
                    psB_p.tile([P, 512], f32, tag="psB", name="px2"),
                    psC_p.tile([P, 512], f32, tag="psC", name="px3"),
                    psA_p.tile([P, 512], f32, tag="psA", name="px4"),
                    psA_p.tile([P, 512], f32, tag="psA", name="px5"),
                    psA_p.tile([P, 512], f32, tag="psA", name="px6"),
                    psR_p.tile([P, 512], f32, tag="psR", name="px7"),
                ]
                for ko in range(KC):
                    for dc in range(KO):
                        nc.tensor.matmul(
                            pxt_ps[dc][:],
                            xnb_t[:, ko, dc * P : (dc + 1) * P],
                            E[:, ko, :],
                            start=(ko == 0),
                            stop=(ko == KC - 1),
                        )
                for dc in range(KO):
                    nc.vector.tensor_copy(pxt_sb[:, dc, :], pxt_ps[dc][:])
                # out[q, e] = PX^T.T @ Wv^T, scaled by 1/rowsum, + bv
                for qs in range(4):
                    pb = psB_p.tile([P, 512], f32, tag="psB", name="avb")
                    pc = psC_p.tile([P, 512], f32, tag="psC", name="avc")
                    for ko in range(KO):
                        lh = pxt_sb[:, ko, qs * P : (qs + 1) * P]
                        nc.tensor.matmul(
                            pb[:], lh, wv_sb[0][:, ko, :],
                            start=(ko == 0), stop=(ko == KO - 1),
                        )
                        nc.tensor.matmul(
                            pc[:], lh, wv_sb[1][:, ko, :],
                            start=(ko == 0), stop=(ko == KO - 1),
                        )
                    row0 = qc * 512 + qs * P
                    for half, ps in ((0, pb), (1, pc)):
                        o = out_pool.tile([P, 512], f32, tag="ost", name="ost")
                        nc.scalar.activation(
                            o[:], ps[:], AF.Identity, scale=recip[:, qs : qs + 1]
                        )
                        nc.vector.tensor_add(
                            o[:], o[:], bvb_t[:, half * 512 : (half + 1) * 512]
                        )
                        nc.sync.dma_start(
                            out[row0 : row0 + P, half * 512 : (half + 1) * 512], o[:]
                        )

    nc.finalize()
    return nc


def make_in_maps(x, Wq, bq, Wk, bk, Wv, bv):
    """Build the 8 per-core input maps from full inputs."""
    import ml_dtypes

    bf16 = ml_dtypes.bfloat16
    x = np.asarray(x, dtype=np.float32)
    # weight-only folding: scores = x (Wq^T Wk) x^T + per-key bias x.(Wk^T bq)
    # (+ per-query consts, softmax-invariant, dropped)
    A = (np.asarray(Wq, np.float64).T @ np.asarray(Wk, np.float64)).astype(np.float32)
    # pre-permute A into per-eo contiguous blocks: rows (eo, ko, p), cols e
    a_pre = np.ascontiguousarray(
        A.reshape(KO, P, KO, P).transpose(2, 0, 1, 3).reshape(KO * D, P)
    )
    wvT = np.ascontiguousarray(np.asarray(Wv, np.float32).T.astype(bf16))
    w3 = (np.asarray(Wk, np.float64).T @ np.asarray(bq, np.float64)).astype(np.float32)
    inv_sqrt_dk = 1.0 / math.sqrt(D // 16)
    bvb = np.ascontiguousarray(np.broadcast_to(np.asarray(bv, np.float32), (P, D)))
    ones_np = np.ones((P, 256), np.float32)

    in_maps = []
    for c in range(N_CORES):
        b, h = c // 2, c % 2
        # key-permute so the core's own query half sits at columns/rows 0..1023
        perm = (
            np.arange(S)
            if h == 0
            else np.concatenate([np.arange(SQ, S), np.arange(0, SQ)])
        )
        xp = x[b][perm]  # [S, D], rows permuted
        t3v = (xp @ w3) * inv_sqrt_dk  # [S]
        in_maps.append(
            {
                "xT": np.ascontiguousarray(xp.T),
                "a": a_pre,
                "xnb": np.ascontiguousarray(xp.astype(bf16)),
                "wvT": wvT,
                "t3": np.ascontiguousarray(t3v.reshape(KC, P).T),
                "ones": ones_np,
                "bvb": bvb,
            }
        )
    return in_maps


_NC_CACHE = None


def get_nc():
    global _NC_CACHE
    if _NC_CACHE is None:
        _NC_CACHE = build_bass()
    return _NC_CACHE


def kernel(x, Wq, bq, Wk, bk, Wv, bv, **run_kwargs):
    from concourse.bass_utils import run_bass_kernel_spmd

    nc = get_nc()
    in_maps = make_in_maps(x, Wq, bq, Wk, bk, Wv, bv)
    res = run_bass_kernel_spmd(
        nc, in_maps, core_ids=list(range(N_CORES)), **run_kwargs
    )
    out = np.empty((B, S, D), dtype=np.float32)
    for c in range(N_CORES):
        b, h = c // 2, c % 2
        out[b, h * SQ : (h + 1) * SQ, :] = res.results[c]["out"]
    if run_kwargs.get("trace"):
        kernel.last_results = res
    return out


# revision 12
# speedup vs baseline: 1.1564x; 1.0019x over previous
"""Full-width attention (B=4, S=2048, D=1024, no head split) on 8 TRN2 cores.

Sharding: data-parallel over (batch, query-half) -> 8 shards. Core c handles
batch b = c//2, query rows [h*1024, (h+1)*1024) with h = c%2.

Zero-redundancy decomposition (12.88 GFLOP/core, the 103 GFLOP/8 floor):
the weight fold Wq^T Wk is applied to the QUERY side, not the key side:
  qm   = x_own A          (A = Wq^T Wk)        2.15 GFLOP   [own 1024 q only]
  S^T  = x_full^T . qm^T  (contract raw e)     4.29 GFLOP   [keys need NO proj]
  E    = exp(S/8 + t3),   t3 = x.(Wk^T bq)     (host-computed, ACT bias)
  PX^T = x^T E            (contract k)         4.29 GFLOP
  out  = (PX/rowsum) Wv^T + bv                 2.15 GFLOP
Per-core inputs are key-permuted (own query half first) so the same SPMD
program can slice "own queries" at columns 0..1023; attention is invariant
to a consistent key permutation of (xT, xnb, t3).

Precision: scores path (A, xT, qm) in fp32r (1-pass FP22, full PE rate).
V path (xn, E, Px, Wv) in bf16 — same PE rate, half the DMA/SBUF, and the
whole value tensor stays SBUF-resident so the PX phase does zero DMA.

Warm-up junk matmuls run off a memset tile (no DMA dependency) so the HAM
clock-gate opens while the first operands stream in.
"""

import math
from contextlib import ExitStack

import numpy as np

P = 128
B, S, D = 4, 2048, 1024
SQ = 1024  # query rows per core
KO = D // P  # 8 chunks of the d/e contraction dims
KC = S // P  # 16 key chunks
N_CORES = 8


def build_bass():
    from concourse import bacc
    import concourse.mybir as mybir
    from concourse.tile import TileContext

    f32 = mybir.dt.float32
    f32r = mybir.dt.float32r
    bf16 = mybir.dt.bfloat16
    AF = mybir.ActivationFunctionType

    nc = bacc.Bacc(
        "TRN2",
        target_bir_lowering=False,
        debug=False,
        enable_asserts=False,
        num_devices=N_CORES,
    )

    xT = nc.dram_tensor("xT", [D, S], f32r, kind="ExternalInput")
    a = nc.dram_tensor("a", [P, KO * D], f32r, kind="ExternalInput")
    xnb = nc.dram_tensor("xnb", [S, D], bf16, kind="ExternalInput")
    wvT = nc.dram_tensor("wvT", [D, D], bf16, kind="ExternalInput")
    t3 = nc.dram_tensor("t3", [P, KC], f32, kind="ExternalInput")
    bvb = nc.dram_tensor("bvb", [P, D], f32, kind="ExternalInput")
    out = nc.dram_tensor("out", [SQ, D], f32, kind="ExternalOutput")

    xT_r = xT[:, :].rearrange("(ko p) s -> p ko s", p=P)
    xnb_r = xnb[:, :].rearrange("(ko p) d -> p ko d", p=P)
    wvT_r = wvT[:, :].rearrange("(ko p) e -> p ko e", p=P)

    inv_sqrt_dk = 1.0 / math.sqrt(D // 16)  # d_key = 64

    with TileContext(nc) as tc, ExitStack() as ctx:
        xt_pool = ctx.enter_context(tc.tile_pool(name="xtp", bufs=1))
        qm_pool = ctx.enter_context(tc.tile_pool(name="qmp", bufs=1))
        msc_pool = ctx.enter_context(tc.tile_pool(name="msc", bufs=1))
        psA_p = ctx.enter_context(tc.tile_pool(name="psA", bufs=3, space="PSUM"))
        psB_p = ctx.enter_context(tc.tile_pool(name="psB", bufs=2, space="PSUM"))
        psC_p = ctx.enter_context(tc.tile_pool(name="psC", bufs=2, space="PSUM"))
        psR_p = ctx.enter_context(tc.tile_pool(name="psR", bufs=1, space="PSUM"))
        dram_p = ctx.enter_context(tc.tile_pool(name="drp", bufs=1, space="DRAM"))

        xt = xt_pool.tile([P, KO, S], f32r)  # raw x^T, resident
        qmT = qm_pool.tile([P, KO, SQ], f32r)  # (x_own A)^T, resident

        # PE warm-up tile generated on-chip (iota + cast): no DMA dependency,
        # so the HAM activity window opens while the first operands stream in.
        iti = msc_pool.tile([P, 256], mybir.dt.int32, tag="iti", name="iti")
        nc.gpsimd.iota(iti[:], pattern=[[0, 256]], base=1, channel_multiplier=0)
        warm = msc_pool.tile([P, 256], f32r, tag="warm", name="warm")
        nc.gpsimd.tensor_copy(warm[:], iti[:])
        t3_t = msc_pool.tile([P, KC], f32, tag="t3t", name="t3_t")
        warm_ps = psR_p.tile([1, 256], f32, tag="psR", name="warm_ps")
        for _ in range(16):
            nc.tensor.matmul(warm_ps[:], warm[:, 0:1], warm[:, :])

        # ---- Phase Q: qm^T[e, q] = A^T x_own^T (A resident, phase-scoped) ----
        with tc.tile_pool(name="ap", bufs=1) as a_pool:
            a_t = a_pool.tile([P, KO * D], f32r)
            for eo in range(KO):
                # host laid A out per-partition-contiguous: 4 KiB bursts
                nc.sync.dma_start(
                    a_t[:, eo * D : (eo + 1) * D], a[:, eo * D : (eo + 1) * D]
                )
            nc.sync.dma_start(t3_t[:], t3[:, :])
            # xT arrives: own-qc0 columns, own-qc1 columns, then the rest
            for ko in range(KO):
                nc.gpsimd.dma_start(xt[:, ko, 0:512], xT_r[:, ko, 0:512])
            for ko in range(KO):
                nc.gpsimd.dma_start(xt[:, ko, 512:1024], xT_r[:, ko, 512:1024])
            for ko in range(KO):
                nc.gpsimd.dma_start(xt[:, ko, 1024:2048], xT_r[:, ko, 1024:2048])

            for qc in range(2):
                for eo in range(KO):
                    pa = psA_p.tile([P, 512], f32, tag="psA", name="paq")
                    for ko in range(KO):
                        nc.tensor.matmul(
                            pa[:],
                            a_t[:, (eo * KO + ko) * P : (eo * KO + ko + 1) * P],
                            xt[:, ko, qc * 512 : (qc + 1) * 512],
                            start=(ko == 0),
                            stop=(ko == KO - 1),
                        )
                    nc.scalar.copy(qmT[:, eo, qc * 512 : (qc + 1) * 512], pa[:])

        # ---------------- Phase C: attention ----------------
        with (
            tc.tile_pool(name="ep", bufs=1) as e_pool,
            tc.tile_pool(name="vsp", bufs=1) as vs_pool,
            tc.tile_pool(name="osp", bufs=2) as out_pool,
        ):
            xnb_t = vs_pool.tile([P, KC, D], bf16, tag="xnb", name="xnb_t")
            for ko in range(KC):
                nc.gpsimd.dma_start(xnb_t[:, ko, :], xnb_r[:, ko, :])
            wv_sb = [
                vs_pool.tile([P, KO, 512], bf16, tag=f"wv{h}", name=f"wv_sb{h}")
                for h in range(2)
            ]
            for h in range(2):
                for ko in range(KO):
                    nc.gpsimd.dma_start(
                        wv_sb[h][:, ko, :], wvT_r[:, ko, h * 512 : (h + 1) * 512]
                    )
            bvb_t = msc_pool.tile([P, D], f32, tag="bvb", name="bvb_t")
            nc.gpsimd.dma_start(bvb_t[:], bvb[:, :])
            pxt_sb = vs_pool.tile([P, KO, 512], bf16, tag="pxt", name="pxt_sb")

            for qc in range(2):
                E = e_pool.tile([P, KC, 512], bf16, tag="E", name="E")
                racc = msc_pool.tile([P, 512], f32r, tag="racc", name="racc")
                for kc in range(KC):
                    pa = psA_p.tile([P, 512], f32, tag="psA", name="pas")
                    for eo in range(KO):
                        nc.tensor.matmul(
                            pa[:],
                            xt[:, eo, kc * P : (kc + 1) * P],
                            qmT[:, eo, qc * 512 : (qc + 1) * 512],
                            start=(eo == 0),
                            stop=(eo == KO - 1),
                        )
                    nc.scalar.activation(
                        E[:, kc, :], pa[:], AF.Exp, scale=inv_sqrt_dk,
                        bias=t3_t[:, kc : kc + 1],
                    )
                    if kc == 0:
                        nc.vector.tensor_copy(racc[:], E[:, 0, :])
                    else:
                        nc.vector.tensor_add(racc[:], racc[:], E[:, kc, :])
                # partition-reduce rowsum with one ones-matmul, then
                # [1,512] -> per-partition recips [128,4] via DRAM bounce
                pr = psR_p.tile([1, 512], f32, tag="psR", name="pr")
                nc.tensor.matmul(pr[:], warm[:, 0:1], racc[:])
                rsum_row = msc_pool.tile([1, 512], f32, tag="rsr", name="rsum_row")
                nc.scalar.copy(rsum_row[:], pr[:])
                rs_dram = dram_p.tile([1, 512], f32, tag="rsd", name="rs_dram")
                nc.sync.dma_start(rs_dram[:, :], rsum_row[:, :])
                rsum_t = msc_pool.tile([P, 4], f32, tag="rst", name="rsum_t")
                nc.sync.dma_start(
                    rsum_t[:, :], rs_dram[0, :].rearrange("(qs p) -> p qs", p=P)
                )
                recip = msc_pool.tile([P, 4], f32, tag="recip", name="recip")
                nc.vector.reciprocal(recip[:], rsum_t[:])

                # PX^T[d, q] = sum_k x[k, d] E[k, q]: fully SBUF-fed (bf16).
                # bank order: outMM consumes psB/psC first, so evac them first
                pxt_ps = [
                    psB_p.tile([P, 512], f32, tag="psB", name="px0"),
                    psC_p.tile([P, 512], f32, tag="psC", name="px1"),
                    psB_p.tile([P, 512], f32, tag="psB", name="px2"),
                    psC_p.tile([P, 512], f32, tag="psC", name="px3"),
                    psA_p.tile([P, 512], f32, tag="psA", name="px4"),
                    psA_p.tile([P, 512], f32, tag="psA", name="px5"),
                    psA_p.tile([P, 512], f32, tag="psA", name="px6"),
                    psR_p.tile([P, 512], f32, tag="psR", name="px7"),
                ]
                for ko in range(KC):
                    for dc in range(KO):
                        nc.tensor.matmul(
                            pxt_ps[dc][:],
                            xnb_t[:, ko, dc * P : (dc + 1) * P],
                            E[:, ko, :],
                            start=(ko == 0),
                            stop=(ko == KC - 1),
                        )
                for dc in range(KO):
                    nc.vector.tensor_copy(pxt_sb[:, dc, :], pxt_ps[dc][:])
                # out[q, e] = PX^T.T @ Wv^T, scaled by 1/rowsum, + bv
                for qs in range(4):
                    pb = psB_p.tile([P, 512], f32, tag="psB", name="avb")
                    pc = psC_p.tile([P, 512], f32, tag="psC", name="avc")
                    for ko in range(KO):
                        lh = pxt_sb[:, ko, qs * P : (qs + 1) * P]
                        nc.tensor.matmul(
                            pb[:], lh, wv_sb[0][:, ko, :],
                            start=(ko == 0), stop=(ko == KO - 1),
                        )
                        nc.tensor.matmul(
                            pc[:], lh, wv_sb[1][:, ko, :],
                            start=(ko == 0), stop=(ko == KO - 1),
                        )
                    row0 = qc * 512 + qs * P
                    for quarter in range(4):
                        ps = pb if quarter < 2 else pc
                        c0 = quarter * 256
                        o = out_pool.tile([P, 256], f32, tag="ost", name="ost")
                        nc.scalar.activation(
                            o[:], ps[:, (quarter % 2) * 256 : (quarter % 2) * 256 + 256],
                            AF.Identity, scale=recip[:, qs : qs + 1],
                        )
                        nc.vector.tensor_add(o[:], o[:], bvb_t[:, c0 : c0 + 256])
                        nc.sync.dma_start(out[row0 : row0 + P, c0 : c0 + 256], o[:])

    nc.finalize()
    return nc


def make_in_maps(x, Wq, bq, Wk, bk, Wv, bv):
    """Build the 8 per-core input maps from full inputs."""
    import ml_dtypes

    bf16 = ml_dtypes.bfloat16
    x = np.asarray(x, dtype=np.float32)
    # weight-only folding: scores = x (Wq^T Wk) x^T + per-key bias x.(Wk^T bq)
    # (+ per-query consts, softmax-invariant, dropped)
    A = (np.asarray(Wq, np.float64).T @ np.asarray(Wk, np.float64)).astype(np.float32)
    # per-partition-contiguous layout: a_pre[p, eo*1024 + ko*128 + e]
    #   = A[ko*128 + p, eo*128 + e] -> each partition reads 4 KiB bursts
    a_pre = np.ascontiguousarray(
        A.reshape(KO, P, KO, P).transpose(1, 2, 0, 3).reshape(P, KO * D)
    )
    wvT = np.ascontiguousarray(np.asarray(Wv, np.float32).T.astype(bf16))
    w3 = (np.asarray(Wk, np.float64).T @ np.asarray(bq, np.float64)).astype(np.float32)
    inv_sqrt_dk = 1.0 / math.sqrt(D // 16)
    bvb = np.ascontiguousarray(np.broadcast_to(np.asarray(bv, np.float32), (P, D)))

    in_maps = []
    for c in range(N_CORES):
        b, h = c // 2, c % 2
        # key-permute so the core's own query half sits at columns/rows 0..1023
        perm = (
            np.arange(S)
            if h == 0
            else np.concatenate([np.arange(SQ, S), np.arange(0, SQ)])
        )
        xp = x[b][perm]  # [S, D], rows permuted
        t3v = (xp @ w3) * inv_sqrt_dk  # [S]
        in_maps.append(
            {
                "xT": np.ascontiguousarray(xp.T),
                "a": a_pre,
                "xnb": np.ascontiguousarray(xp.astype(bf16)),
                "wvT": wvT,
                "t3": np.ascontiguousarray(t3v.reshape(KC, P).T),
                "bvb": bvb,
            }
        )
    return in_maps


_NC_CACHE = None


def get_nc():
    global _NC_CACHE
    if _NC_CACHE is None:
        _NC_CACHE = build_bass()
    return _NC_CACHE


def kernel(x, Wq, bq, Wk, bk, Wv, bv, **run_kwargs):
    from concourse.bass_utils import run_bass_kernel_spmd

    nc = get_nc()
    in_maps = make_in_maps(x, Wq, bq, Wk, bk, Wv, bv)
    res = run_bass_kernel_spmd(
        nc, in_maps, core_ids=list(range(N_CORES)), **run_kwargs
    )
    out = np.empty((B, S, D), dtype=np.float32)
    for c in range(N_CORES):
        b, h = c // 2, c % 2
        out[b, h * SQ : (h + 1) * SQ, :] = res.results[c]["out"]
    if run_kwargs.get("trace"):
        kernel.last_results = res
    return out


# revision 14
# speedup vs baseline: 1.2484x; 1.0796x over previous
"""Full-width attention (B=4, S=2048, D=1024, no head split) on 8 TRN2 cores.

Sharding: data-parallel over (batch, query-half) -> 8 shards. Core c handles
batch b = c//2, query rows [h*1024, (h+1)*1024) with h = c%2.

Zero-redundancy decomposition (12.88 GFLOP/core, the 103 GFLOP/8 floor):
the weight fold Wq^T Wk is applied to the QUERY side, not the key side:
  qm   = x_own A          (A = Wq^T Wk)        2.15 GFLOP   [own 1024 q only]
  S^T  = x_full^T . qm^T  (contract raw e)     4.29 GFLOP   [keys need NO proj]
  E    = exp(S/8 + t3),   t3 = x.(Wk^T bq)     (host-computed, ACT bias)
  PX^T = x^T E            (contract k)         4.29 GFLOP
  out  = (PX/rowsum) Wv^T + bv                 2.15 GFLOP
Per-core inputs are key-permuted (own query half first) so the same SPMD
program can slice "own queries" at columns 0..1023; attention is invariant
to a consistent key permutation of (xT, xnb, t3).

Precision: scores path (A, xT, qm) in fp32r (1-pass FP22, full PE rate).
V path (xn, E, Px, Wv) in bf16 — same PE rate, half the DMA/SBUF, and the
whole value tensor stays SBUF-resident so the PX phase does zero DMA.

Warm-up junk matmuls run off a memset tile (no DMA dependency) so the HAM
clock-gate opens while the first operands stream in.
"""

import math
from contextlib import ExitStack

import numpy as np

P = 128
B, S, D = 4, 2048, 1024
SQ = 1024  # query rows per core
KO = D // P  # 8 chunks of the d/e contraction dims
KC = S // P  # 16 key chunks
N_CORES = 8


def build_bass():
    from concourse import bacc
    import concourse.mybir as mybir
    from concourse.tile import TileContext

    f32 = mybir.dt.float32
    f32r = mybir.dt.float32r
    bf16 = mybir.dt.bfloat16
    AF = mybir.ActivationFunctionType

    nc = bacc.Bacc(
        "TRN2",
        target_bir_lowering=False,
        debug=False,
        enable_asserts=False,
        num_devices=N_CORES,
    )

    xT = nc.dram_tensor("xT", [D, S], f32r, kind="ExternalInput")
    a = nc.dram_tensor("a", [P, KO * D], f32r, kind="ExternalInput")
    xnb = nc.dram_tensor("xnb", [S, D], bf16, kind="ExternalInput")
    wvT = nc.dram_tensor("wvT", [D, D], bf16, kind="ExternalInput")
    t3 = nc.dram_tensor("t3", [P, KC], f32, kind="ExternalInput")
    bvb = nc.dram_tensor("bvb", [P, D], f32, kind="ExternalInput")
    out = nc.dram_tensor("out", [SQ, D], f32, kind="ExternalOutput")

    xT_r = xT[:, :].rearrange("(ko p) s -> p ko s", p=P)
    xnb_r = xnb[:, :].rearrange("(ko p) d -> p ko d", p=P)
    wvT_r = wvT[:, :].rearrange("(ko p) e -> p ko e", p=P)

    inv_sqrt_dk = 1.0 / math.sqrt(D // 16)  # d_key = 64

    with TileContext(nc) as tc, ExitStack() as ctx:
        xt_pool = ctx.enter_context(tc.tile_pool(name="xtp", bufs=1))
        qm_pool = ctx.enter_context(tc.tile_pool(name="qmp", bufs=1))
        msc_pool = ctx.enter_context(tc.tile_pool(name="msc", bufs=1))
        psA_p = ctx.enter_context(tc.tile_pool(name="psA", bufs=3, space="PSUM"))
        psB_p = ctx.enter_context(tc.tile_pool(name="psB", bufs=2, space="PSUM"))
        psC_p = ctx.enter_context(tc.tile_pool(name="psC", bufs=2, space="PSUM"))
        psR_p = ctx.enter_context(tc.tile_pool(name="psR", bufs=1, space="PSUM"))
        dram_p = ctx.enter_context(tc.tile_pool(name="drp", bufs=1, space="DRAM"))

        xt = xt_pool.tile([P, KO, S], f32r)  # raw x^T, resident
        qmT = qm_pool.tile([P, KO, SQ], f32r)  # (x_own A)^T, resident

        # PE warm-up tile generated on-chip (iota + cast): no DMA dependency,
        # so the HAM activity window opens while the first operands stream in.
        iti = msc_pool.tile([P, 256], mybir.dt.int32, tag="iti", name="iti")
        nc.gpsimd.iota(iti[:], pattern=[[0, 256]], base=1, channel_multiplier=0)
        warm = msc_pool.tile([P, 256], f32r, tag="warm", name="warm")
        nc.gpsimd.tensor_copy(warm[:], iti[:])
        t3_t = msc_pool.tile([P, KC], f32, tag="t3t", name="t3_t")
        warm_ps = psR_p.tile([1, 256], f32, tag="psR", name="warm_ps")
        for _ in range(16):
            nc.tensor.matmul(warm_ps[:], warm[:, 0:1], warm[:, :])

        # ---- Phase Q: qm^T[e, q] = A^T x_own^T (A resident, phase-scoped) ----
        with tc.tile_pool(name="ap", bufs=1) as a_pool:
            a_t = a_pool.tile([P, KO * D], f32r)
            # first qm iteration needs a[eo=0] plus ALL qc0 columns of xT:
            # interleave those across both queues so they land by ~10.5us,
            # then stream the per-eo A blocks just-in-time on sync.
            nc.sync.dma_start(a_t[:, 0:D], a[:, 0:D])
            for ko in range(KO):
                q = nc.sync if ko % 2 == 0 else nc.gpsimd
                q.dma_start(xt[:, ko, 0:512], xT_r[:, ko, 0:512])
            for eo in range(1, KO):
                nc.sync.dma_start(
                    a_t[:, eo * D : (eo + 1) * D], a[:, eo * D : (eo + 1) * D]
                )
            nc.sync.dma_start(t3_t[:], t3[:, :])
            # own-qc1 columns, then the non-own key columns
            for ko in range(KO):
                nc.gpsimd.dma_start(xt[:, ko, 512:1024], xT_r[:, ko, 512:1024])
            for ko in range(KO):
                nc.gpsimd.dma_start(xt[:, ko, 1024:2048], xT_r[:, ko, 1024:2048])

            for qc in range(2):
                for eo in range(KO):
                    pa = psA_p.tile([P, 512], f32, tag="psA", name="paq")
                    for ko in range(KO):
                        nc.tensor.matmul(
                            pa[:],
                            a_t[:, (eo * KO + ko) * P : (eo * KO + ko + 1) * P],
                            xt[:, ko, qc * 512 : (qc + 1) * 512],
                            start=(ko == 0),
                            stop=(ko == KO - 1),
                        )
                    nc.scalar.copy(qmT[:, eo, qc * 512 : (qc + 1) * 512], pa[:])

        # ---------------- Phase C: attention ----------------
        with (
            tc.tile_pool(name="ep", bufs=1) as e_pool,
            tc.tile_pool(name="vsp", bufs=1) as vs_pool,
            tc.tile_pool(name="osp", bufs=12) as out_pool,
        ):
            xnb_t = vs_pool.tile([P, KC, D], bf16, tag="xnb", name="xnb_t")
            for ko in range(KC):
                nc.gpsimd.dma_start(xnb_t[:, ko, :], xnb_r[:, ko, :])
            wv_sb = [
                vs_pool.tile([P, KO, 512], bf16, tag=f"wv{h}", name=f"wv_sb{h}")
                for h in range(2)
            ]
            for h in range(2):
                for ko in range(KO):
                    nc.gpsimd.dma_start(
                        wv_sb[h][:, ko, :], wvT_r[:, ko, h * 512 : (h + 1) * 512]
                    )
            bvb_t = msc_pool.tile([P, D], f32, tag="bvb", name="bvb_t")
            nc.gpsimd.dma_start(bvb_t[:], bvb[:, :])
            pxt_sb = vs_pool.tile([P, KO, 512], bf16, tag="pxt", name="pxt_sb")

            for qc in range(2):
                E = e_pool.tile([P, KC, 512], bf16, tag="E", name="E")
                racc = msc_pool.tile([P, 512], f32r, tag="racc", name="racc")
                for kc in range(KC):
                    pa = psA_p.tile([P, 512], f32, tag="psA", name="pas")
                    for eo in range(KO):
                        nc.tensor.matmul(
                            pa[:],
                            xt[:, eo, kc * P : (kc + 1) * P],
                            qmT[:, eo, qc * 512 : (qc + 1) * 512],
                            start=(eo == 0),
                            stop=(eo == KO - 1),
                        )
                    nc.scalar.activation(
                        E[:, kc, :], pa[:], AF.Exp, scale=inv_sqrt_dk,
                        bias=t3_t[:, kc : kc + 1],
                    )
                    if kc == 0:
                        nc.vector.tensor_copy(racc[:], E[:, 0, :])
                    else:
                        nc.vector.tensor_add(racc[:], racc[:], E[:, kc, :])
                # partition-reduce rowsum with one ones-matmul, then
                # [1,512] -> per-partition recips [128,4] via DRAM bounce
                pr = psR_p.tile([1, 512], f32, tag="psR", name="pr")
                nc.tensor.matmul(pr[:], warm[:, 0:1], racc[:])
                rsum_row = msc_pool.tile([1, 512], f32, tag="rsr", name="rsum_row")
                nc.scalar.copy(rsum_row[:], pr[:])
                rs_dram = dram_p.tile([1, 512], f32, tag="rsd", name="rs_dram")
                nc.sync.dma_start(rs_dram[:, :], rsum_row[:, :])
                rsum_t = msc_pool.tile([P, 4], f32, tag="rst", name="rsum_t")
                nc.sync.dma_start(
                    rsum_t[:, :], rs_dram[0, :].rearrange("(qs p) -> p qs", p=P)
                )
                recip = msc_pool.tile([P, 4], f32, tag="recip", name="recip")
                nc.vector.reciprocal(recip[:], rsum_t[:])

                # PX^T[d, q] = sum_k x[k, d] E[k, q]: fully SBUF-fed (bf16).
                # bank order: outMM consumes psB/psC first, so evac them first
                pxt_ps = [
                    psB_p.tile([P, 512], f32, tag="psB", name="px0"),
                    psC_p.tile([P, 512], f32, tag="psC", name="px1"),
                    psB_p.tile([P, 512], f32, tag="psB", name="px2"),
                    psC_p.tile([P, 512], f32, tag="psC", name="px3"),
                    psA_p.tile([P, 512], f32, tag="psA", name="px4"),
                    psA_p.tile([P, 512], f32, tag="psA", name="px5"),
                    psA_p.tile([P, 512], f32, tag="psA", name="px6"),
                    psR_p.tile([P, 512], f32, tag="psR", name="px7"),
                ]
                for ko in range(KC):
                    for dc in range(KO):
                        nc.tensor.matmul(
                            pxt_ps[dc][:],
                            xnb_t[:, ko, dc * P : (dc + 1) * P],
                            E[:, ko, :],
                            start=(ko == 0),
                            stop=(ko == KC - 1),
                        )
                for dc in range(KO):
                    nc.vector.tensor_copy(pxt_sb[:, dc, :], pxt_ps[dc][:])
                # out[q, e] = PX^T.T @ Wv^T, scaled by 1/rowsum, + bv
                for qs in range(4):
                    pb = psB_p.tile([P, 512], f32, tag="psB", name="avb")
                    pc = psC_p.tile([P, 512], f32, tag="psC", name="avc")
                    for ko in range(KO):
                        lh = pxt_sb[:, ko, qs * P : (qs + 1) * P]
                        nc.tensor.matmul(
                            pb[:], lh, wv_sb[0][:, ko, :],
                            start=(ko == 0), stop=(ko == KO - 1),
                        )
                        nc.tensor.matmul(
                            pc[:], lh, wv_sb[1][:, ko, :],
                            start=(ko == 0), stop=(ko == KO - 1),
                        )
                    row0 = qc * 512 + qs * P
                    for quarter in range(4):
                        ps = pb if quarter < 2 else pc
                        c0 = quarter * 256
                        o = out_pool.tile([P, 256], f32, tag="ost", name="ost")
                        nc.scalar.activation(
                            o[:], ps[:, (quarter % 2) * 256 : (quarter % 2) * 256 + 256],
                            AF.Identity, scale=recip[:, qs : qs + 1],
                        )
                        nc.vector.tensor_add(o[:], o[:], bvb_t[:, c0 : c0 + 256])
                        nc.sync.dma_start(out[row0 : row0 + P, c0 : c0 + 256], o[:])

    nc.finalize()
    return nc


def make_in_maps(x, Wq, bq, Wk, bk, Wv, bv):
    """Build the 8 per-core input maps from full inputs."""
    import ml_dtypes

    bf16 = ml_dtypes.bfloat16
    x = np.asarray(x, dtype=np.float32)
    # weight-only folding: scores = x (Wq^T Wk) x^T + per-key bias x.(Wk^T bq)
    # (+ per-query consts, softmax-invariant, dropped)
    A = (np.asarray(Wq, np.float64).T @ np.asarray(Wk, np.float64)).astype(np.float32)
    # per-partition-contiguous layout: a_pre[p, eo*1024 + ko*128 + e]
    #   = A[ko*128 + p, eo*128 + e] -> each partition reads 4 KiB bursts
    a_pre = np.ascontiguousarray(
        A.reshape(KO, P, KO, P).transpose(1, 2, 0, 3).reshape(P, KO * D)
    )
    wvT = np.ascontiguousarray(np.asarray(Wv, np.float32).T.astype(bf16))
    w3 = (np.asarray(Wk, np.float64).T @ np.asarray(bq, np.float64)).astype(np.float32)
    inv_sqrt_dk = 1.0 / math.sqrt(D // 16)
    bvb = np.ascontiguousarray(np.broadcast_to(np.asarray(bv, np.float32), (P, D)))

    in_maps = []
    for c in range(N_CORES):
        b, h = c // 2, c % 2
        # key-permute so the core's own query half sits at columns/rows 0..1023
        perm = (
            np.arange(S)
            if h == 0
            else np.concatenate([np.arange(SQ, S), np.arange(0, SQ)])
        )
        xp = x[b][perm]  # [S, D], rows permuted
        t3v = (xp @ w3) * inv_sqrt_dk  # [S]
        in_maps.append(
            {
                "xT": np.ascontiguousarray(xp.T),
                "a": a_pre,
                "xnb": np.ascontiguousarray(xp.astype(bf16)),
                "wvT": wvT,
                "t3": np.ascontiguousarray(t3v.reshape(KC, P).T),
                "bvb": bvb,
            }
        )
    return in_maps


_NC_CACHE = None


def get_nc():
    global _NC_CACHE
    if _NC_CACHE is None:
        _NC_CACHE = build_bass()
    return _NC_CACHE


def kernel(x, Wq, bq, Wk, bk, Wv, bv, **run_kwargs):
    from concourse.bass_utils import run_bass_kernel_spmd

    nc = get_nc()
    in_maps = make_in_maps(x, Wq, bq, Wk, bk, Wv, bv)
    res = run_bass_kernel_spmd(
        nc, in_maps, core_ids=list(range(N_CORES)), **run_kwargs
    )
    out = np.empty((B, S, D), dtype=np.float32)
    for c in range(N_CORES):
        b, h = c // 2, c % 2
        out[b, h * SQ : (h + 1) * SQ, :] = res.results[c]["out"]
    if run_kwargs.get("trace"):
        kernel.last_results = res
    return out


# revision 16
# speedup vs baseline: 1.2509x; 1.0019x over previous
"""Full-width attention (B=4, S=2048, D=1024, no head split) on 8 TRN2 cores.

Sharding: data-parallel over (batch, query-half) -> 8 shards. Core c handles
batch b = c//2, query rows [h*1024, (h+1)*1024) with h = c%2.

Zero-redundancy decomposition (12.88 GFLOP/core, the 103 GFLOP/8 floor):
the weight fold Wq^T Wk is applied to the QUERY side, not the key side:
  qm   = x_own A          (A = Wq^T Wk)        2.15 GFLOP   [own 1024 q only]
  S^T  = x_full^T . qm^T  (contract raw e)     4.29 GFLOP   [keys need NO proj]
  E    = exp(S/8 + t3),   t3 = x.(Wk^T bq)     (host-computed, ACT bias)
  PX^T = x^T E            (contract k)         4.29 GFLOP
  out  = (PX/rowsum) Wv^T + bv                 2.15 GFLOP
Per-core inputs are key-permuted (own query half first) so the same SPMD
program can slice "own queries" at columns 0..1023; attention is invariant
to a consistent key permutation of (xT, xnb, t3).

Precision: scores path (A, xT, qm) in fp32r (1-pass FP22, full PE rate).
V path (xn, E, Px, Wv) in bf16 — same PE rate, half the DMA/SBUF, and the
whole value tensor stays SBUF-resident so the PX phase does zero DMA.

Warm-up junk matmuls run off a memset tile (no DMA dependency) so the HAM
clock-gate opens while the first operands stream in.
"""

import math
from contextlib import ExitStack

import numpy as np

P = 128
B, S, D = 4, 2048, 1024
SQ = 1024  # query rows per core
KO = D // P  # 8 chunks of the d/e contraction dims
KC = S // P  # 16 key chunks
N_CORES = 8


def build_bass():
    from concourse import bacc
    import concourse.mybir as mybir
    from concourse.tile import TileContext

    f32 = mybir.dt.float32
    f32r = mybir.dt.float32r
    bf16 = mybir.dt.bfloat16
    AF = mybir.ActivationFunctionType

    nc = bacc.Bacc(
        "TRN2",
        target_bir_lowering=False,
        debug=False,
        enable_asserts=False,
        num_devices=N_CORES,
    )

    xT = nc.dram_tensor("xT", [D, S], f32r, kind="ExternalInput")
    a = nc.dram_tensor("a", [P, KO * D], f32r, kind="ExternalInput")
    xnb = nc.dram_tensor("xnb", [S, D], bf16, kind="ExternalInput")
    wvT = nc.dram_tensor("wvT", [D, D], bf16, kind="ExternalInput")
    t3 = nc.dram_tensor("t3", [P, KC], f32, kind="ExternalInput")
    bvb = nc.dram_tensor("bvb", [P, D], f32, kind="ExternalInput")
    out = nc.dram_tensor("out", [SQ, D], f32, kind="ExternalOutput")

    xT_r = xT[:, :].rearrange("(ko p) s -> p ko s", p=P)
    xnb_r = xnb[:, :].rearrange("(ko p) d -> p ko d", p=P)
    wvT_r = wvT[:, :].rearrange("(ko p) e -> p ko e", p=P)

    inv_sqrt_dk = 1.0 / math.sqrt(D // 16)  # d_key = 64

    with TileContext(nc) as tc, ExitStack() as ctx:
        xt_pool = ctx.enter_context(tc.tile_pool(name="xtp", bufs=1))
        qm_pool = ctx.enter_context(tc.tile_pool(name="qmp", bufs=1))
        msc_pool = ctx.enter_context(tc.tile_pool(name="msc", bufs=1))
        psA_p = ctx.enter_context(tc.tile_pool(name="psA", bufs=3, space="PSUM"))
        psB_p = ctx.enter_context(tc.tile_pool(name="psB", bufs=2, space="PSUM"))
        psC_p = ctx.enter_context(tc.tile_pool(name="psC", bufs=2, space="PSUM"))
        psR_p = ctx.enter_context(tc.tile_pool(name="psR", bufs=1, space="PSUM"))
        dram_p = ctx.enter_context(tc.tile_pool(name="drp", bufs=1, space="DRAM"))

        xt = xt_pool.tile([P, KO, S], f32r)  # raw x^T, resident
        qmT = qm_pool.tile([P, KO, SQ], f32r)  # (x_own A)^T, resident

        # PE warm-up tile generated on-chip (iota + cast): no DMA dependency,
        # so the HAM activity window opens while the first operands stream in.
        iti = msc_pool.tile([P, 256], mybir.dt.int32, tag="iti", name="iti")
        nc.gpsimd.iota(iti[:], pattern=[[0, 256]], base=1, channel_multiplier=0)
        warm = msc_pool.tile([P, 256], f32r, tag="warm", name="warm")
        nc.vector.tensor_copy(warm[:], iti[:])
        t3_t = msc_pool.tile([P, KC], f32, tag="t3t", name="t3_t")
        warm_ps = psR_p.tile([1, 256], f32, tag="psR", name="warm_ps")
        for _ in range(16):
            nc.tensor.matmul(warm_ps[:], warm[:, 0:1], warm[:, :])

        # ---- Phase Q: qm^T[e, q] = A^T x_own^T (A resident, phase-scoped) ----
        with tc.tile_pool(name="ap", bufs=1) as a_pool:
            a_t = a_pool.tile([P, KO * D], f32r)
            # first qm iteration needs a[eo=0] plus ALL qc0 columns of xT:
            # interleave those across both queues so they land by ~10.5us,
            # then stream the per-eo A blocks just-in-time on sync.
            nc.sync.dma_start(a_t[:, 0:D], a[:, 0:D])
            for ko in range(KO):
                q = nc.sync if ko % 2 == 0 else nc.gpsimd
                q.dma_start(xt[:, ko, 0:512], xT_r[:, ko, 0:512])
            for eo in range(1, KO):
                nc.sync.dma_start(
                    a_t[:, eo * D : (eo + 1) * D], a[:, eo * D : (eo + 1) * D]
                )
            nc.sync.dma_start(t3_t[:], t3[:, :])
            # own-qc1 columns, then the non-own key columns
            for ko in range(KO):
                nc.gpsimd.dma_start(xt[:, ko, 512:1024], xT_r[:, ko, 512:1024])
            for ko in range(KO):
                nc.gpsimd.dma_start(xt[:, ko, 1024:2048], xT_r[:, ko, 1024:2048])

            for qc in range(2):
                for eo in range(KO):
                    pa = psA_p.tile([P, 512], f32, tag="psA", name="paq")
                    for ko in range(KO):
                        nc.tensor.matmul(
                            pa[:],
                            a_t[:, (eo * KO + ko) * P : (eo * KO + ko + 1) * P],
                            xt[:, ko, qc * 512 : (qc + 1) * 512],
                            start=(ko == 0),
                            stop=(ko == KO - 1),
                        )
                    nc.scalar.copy(qmT[:, eo, qc * 512 : (qc + 1) * 512], pa[:])

        # ---------------- Phase C: attention ----------------
        with (
            tc.tile_pool(name="ep", bufs=1) as e_pool,
            tc.tile_pool(name="vsp", bufs=1) as vs_pool,
            tc.tile_pool(name="osp", bufs=12) as out_pool,
        ):
            xnb_t = vs_pool.tile([P, KC, D], bf16, tag="xnb", name="xnb_t")
            for ko in range(KC):
                nc.gpsimd.dma_start(xnb_t[:, ko, :], xnb_r[:, ko, :])
            wv_sb = [
                vs_pool.tile([P, KO, 512], bf16, tag=f"wv{h}", name=f"wv_sb{h}")
                for h in range(2)
            ]
            for h in range(2):
                for ko in range(KO):
                    nc.gpsimd.dma_start(
                        wv_sb[h][:, ko, :], wvT_r[:, ko, h * 512 : (h + 1) * 512]
                    )
            bvb_t = msc_pool.tile([P, D], f32, tag="bvb", name="bvb_t")
            nc.gpsimd.dma_start(bvb_t[:], bvb[:, :])
            pxt_sb = vs_pool.tile([P, KO, 512], bf16, tag="pxt", name="pxt_sb")

            for qc in range(2):
                E = e_pool.tile([P, KC, 512], bf16, tag="E", name="E")
                racc = msc_pool.tile([P, 512], f32r, tag="racc", name="racc")
                for kc in range(KC):
                    pa = psA_p.tile([P, 512], f32, tag="psA", name="pas")
                    for eo in range(KO):
                        nc.tensor.matmul(
                            pa[:],
                            xt[:, eo, kc * P : (kc + 1) * P],
                            qmT[:, eo, qc * 512 : (qc + 1) * 512],
                            start=(eo == 0),
                            stop=(eo == KO - 1),
                        )
                    nc.scalar.activation(
                        E[:, kc, :], pa[:], AF.Exp, scale=inv_sqrt_dk,
                        bias=t3_t[:, kc : kc + 1],
                    )
                    if kc == 0:
                        nc.vector.tensor_copy(racc[:], E[:, 0, :])
                    else:
                        nc.vector.tensor_add(racc[:], racc[:], E[:, kc, :])
                # partition-reduce rowsum with one ones-matmul, then
                # [1,512] -> per-partition recips [128,4] via DRAM bounce
                pr = psR_p.tile([1, 512], f32, tag="psR", name="pr")
                nc.tensor.matmul(pr[:], warm[:, 0:1], racc[:])
                rsum_row = msc_pool.tile([1, 512], f32, tag="rsr", name="rsum_row")
                nc.scalar.copy(rsum_row[:], pr[:])
                rs_dram = dram_p.tile([1, 512], f32, tag="rsd", name="rs_dram")
                nc.sync.dma_start(rs_dram[:, :], rsum_row[:, :])
                rsum_t = msc_pool.tile([P, 4], f32, tag="rst", name="rsum_t")
                nc.sync.dma_start(
                    rsum_t[:, :], rs_dram[0, :].rearrange("(qs p) -> p qs", p=P)
                )
                recip = msc_pool.tile([P, 4], f32, tag="recip", name="recip")
                nc.vector.reciprocal(recip[:], rsum_t[:])

                # PX^T[d, q] = sum_k x[k, d] E[k, q]: fully SBUF-fed (bf16).
                # bank order: outMM consumes psB/psC first, so evac them first
                pxt_ps = [
                    psB_p.tile([P, 512], f32, tag="psB", name="px0"),
                    psC_p.tile([P, 512], f32, tag="psC", name="px1"),
                    psB_p.tile([P, 512], f32, tag="psB", name="px2"),
                    psC_p.tile([P, 512], f32, tag="psC", name="px3"),
                    psA_p.tile([P, 512], f32, tag="psA", name="px4"),
                    psA_p.tile([P, 512], f32, tag="psA", name="px5"),
                    psA_p.tile([P, 512], f32, tag="psA", name="px6"),
                    psR_p.tile([P, 512], f32, tag="psR", name="px7"),
                ]
                for ko in range(KC):
                    for dc in range(KO):
                        nc.tensor.matmul(
                            pxt_ps[dc][:],
                            xnb_t[:, ko, dc * P : (dc + 1) * P],
                            E[:, ko, :],
                            start=(ko == 0),
                            stop=(ko == KC - 1),
                        )
                for dc in range(KO):
                    nc.vector.tensor_copy(pxt_sb[:, dc, :], pxt_ps[dc][:])
                # out[q, e] = PX^T.T @ Wv^T, scaled by 1/rowsum, + bv
                for qs in range(4):
                    pb = psB_p.tile([P, 512], f32, tag="psB", name="avb")
                    pc = psC_p.tile([P, 512], f32, tag="psC", name="avc")
                    for ko in range(KO):
                        lh = pxt_sb[:, ko, qs * P : (qs + 1) * P]
                        nc.tensor.matmul(
                            pb[:], lh, wv_sb[0][:, ko, :],
                            start=(ko == 0), stop=(ko == KO - 1),
                        )
                        nc.tensor.matmul(
                            pc[:], lh, wv_sb[1][:, ko, :],
                            start=(ko == 0), stop=(ko == KO - 1),
                        )
                    row0 = qc * 512 + qs * P
                    for quarter in range(4):
                        ps = pb if quarter < 2 else pc
                        c0 = quarter * 256
                        o = out_pool.tile([P, 256], f32, tag="ost", name="ost")
                        nc.scalar.activation(
                            o[:], ps[:, (quarter % 2) * 256 : (quarter % 2) * 256 + 256],
                            AF.Identity, scale=recip[:, qs : qs + 1],
                        )
                        nc.vector.tensor_add(o[:], o[:], bvb_t[:, c0 : c0 + 256])
                        nc.sync.dma_start(out[row0 : row0 + P, c0 : c0 + 256], o[:])

    nc.finalize()
    return nc


def make_in_maps(x, Wq, bq, Wk, bk, Wv, bv):
    """Build the 8 per-core input maps from full inputs."""
    import ml_dtypes

    bf16 = ml_dtypes.bfloat16
    x = np.asarray(x, dtype=np.float32)
    # weight-only folding: scores = x (Wq^T Wk) x^T + per-key bias x.(Wk^T bq)
    # (+ per-query consts, softmax-invariant, dropped)
    A = (np.asarray(Wq, np.float64).T @ np.asarray(Wk, np.float64)).astype(np.float32)
    # per-partition-contiguous layout: a_pre[p, eo*1024 + ko*128 + e]
    #   = A[ko*128 + p, eo*128 + e] -> each partition reads 4 KiB bursts
    a_pre = np.ascontiguousarray(
        A.reshape(KO, P, KO, P).transpose(1, 2, 0, 3).reshape(P, KO * D)
    )
    wvT = np.ascontiguousarray(np.asarray(Wv, np.float32).T.astype(bf16))
    w3 = (np.asarray(Wk, np.float64).T @ np.asarray(bq, np.float64)).astype(np.float32)
    inv_sqrt_dk = 1.0 / math.sqrt(D // 16)
    bvb = np.ascontiguousarray(np.broadcast_to(np.asarray(bv, np.float32), (P, D)))

    in_maps = []
    for c in range(N_CORES):
        b, h = c // 2, c % 2
        # key-permute so the core's own query half sits at columns/rows 0..1023
        perm = (
            np.arange(S)
            if h == 0
            else np.concatenate([np.arange(SQ, S), np.arange(0, SQ)])
        )
        xp = x[b][perm]  # [S, D], rows permuted
        t3v = (xp @ w3) * inv_sqrt_dk  # [S]
        in_maps.append(
            {
                "xT": np.ascontiguousarray(xp.T),
                "a": a_pre,
                "xnb": np.ascontiguousarray(xp.astype(bf16)),
                "wvT": wvT,
                "t3": np.ascontiguousarray(t3v.reshape(KC, P).T),
                "bvb": bvb,
            }
        )
    return in_maps


_NC_CACHE = None


def get_nc():
    global _NC_CACHE
    if _NC_CACHE is None:
        _NC_CACHE = build_bass()
    return _NC_CACHE


def kernel(x, Wq, bq, Wk, bk, Wv, bv, **run_kwargs):
    from concourse.bass_utils import run_bass_kernel_spmd

    nc = get_nc()
    in_maps = make_in_maps(x, Wq, bq, Wk, bk, Wv, bv)
    res = run_bass_kernel_spmd(
        nc, in_maps, core_ids=list(range(N_CORES)), **run_kwargs
    )
    out = np.empty((B, S, D), dtype=np.float32)
    for c in range(N_CORES):
        b, h = c // 2, c % 2
        out[b, h * SQ : (h + 1) * SQ, :] = res.results[c]["out"]
    if run_kwargs.get("trace"):
        kernel.last_results = res
    return out


# revision 17
# speedup vs baseline: 1.2697x; 1.0150x over previous
"""Full-width attention (B=4, S=2048, D=1024, no head split) on 8 TRN2 cores.

Sharding: data-parallel over (batch, query-half) -> 8 shards. Core c handles
batch b = c//2, query rows [h*1024, (h+1)*1024) with h = c%2.

Zero-redundancy decomposition (12.88 GFLOP/core, the 103 GFLOP/8 floor):
the weight fold Wq^T Wk is applied to the QUERY side, not the key side:
  qm   = x_own A          (A = Wq^T Wk)        2.15 GFLOP   [own 1024 q only]
  S^T  = x_full^T . qm^T  (contract raw e)     4.29 GFLOP   [keys need NO proj]
  E    = exp(S/8 + t3),   t3 = x.(Wk^T bq)     (host-computed, ACT bias)
  PX^T = x^T E            (contract k)         4.29 GFLOP
  out  = (PX/rowsum) Wv^T + bv                 2.15 GFLOP
Per-core inputs are key-permuted (own query half first) so the same SPMD
program can slice "own queries" at columns 0..1023; attention is invariant
to a consistent key permutation of (xT, xnb, t3).

Precision: scores path (A, xT, qm) in fp32r (1-pass FP22, full PE rate).
V path (xn, E, Px, Wv) in bf16 — same PE rate, half the DMA/SBUF, and the
whole value tensor stays SBUF-resident so the PX phase does zero DMA.

Warm-up junk matmuls run off a memset tile (no DMA dependency) so the HAM
clock-gate opens while the first operands stream in.
"""

import math
from contextlib import ExitStack

import numpy as np

P = 128
B, S, D = 4, 2048, 1024
SQ = 1024  # query rows per core
KO = D // P  # 8 chunks of the d/e contraction dims
KC = S // P  # 16 key chunks
N_CORES = 8


def build_bass():
    from concourse import bacc
    import concourse.mybir as mybir
    from concourse.tile import TileContext

    f32 = mybir.dt.float32
    f32r = mybir.dt.float32r
    bf16 = mybir.dt.bfloat16
    AF = mybir.ActivationFunctionType

    nc = bacc.Bacc(
        "TRN2",
        target_bir_lowering=False,
        debug=False,
        enable_asserts=False,
        num_devices=N_CORES,
    )

    xT = nc.dram_tensor("xT", [D, S], f32r, kind="ExternalInput")
    a = nc.dram_tensor("a", [P, KO * D], f32r, kind="ExternalInput")
    xnb = nc.dram_tensor("xnb", [S, D], bf16, kind="ExternalInput")
    wvT = nc.dram_tensor("wvT", [D, D], bf16, kind="ExternalInput")
    t3 = nc.dram_tensor("t3", [P, KC], f32, kind="ExternalInput")
    bvb = nc.dram_tensor("bvb", [P, D], f32, kind="ExternalInput")
    out = nc.dram_tensor("out", [SQ, D], f32, kind="ExternalOutput")

    xT_r = xT[:, :].rearrange("(ko p) s -> p ko s", p=P)
    xnb_r = xnb[:, :].rearrange("(ko p) d -> p ko d", p=P)
    wvT_r = wvT[:, :].rearrange("(ko p) e -> p ko e", p=P)

    inv_sqrt_dk = 1.0 / math.sqrt(D // 16)  # d_key = 64

    with TileContext(nc) as tc, ExitStack() as ctx:
        xt_pool = ctx.enter_context(tc.tile_pool(name="xtp", bufs=1))
        qm_pool = ctx.enter_context(tc.tile_pool(name="qmp", bufs=1))
        msc_pool = ctx.enter_context(tc.tile_pool(name="msc", bufs=1))
        psA_p = ctx.enter_context(tc.tile_pool(name="psA", bufs=3, space="PSUM"))
        psB_p = ctx.enter_context(tc.tile_pool(name="psB", bufs=2, space="PSUM"))
        psC_p = ctx.enter_context(tc.tile_pool(name="psC", bufs=2, space="PSUM"))
        psR_p = ctx.enter_context(tc.tile_pool(name="psR", bufs=1, space="PSUM"))
        dram_p = ctx.enter_context(tc.tile_pool(name="drp", bufs=1, space="DRAM"))

        xt = xt_pool.tile([P, KO, S], f32r)  # raw x^T, resident
        qmT = qm_pool.tile([P, KO, SQ], f32r)  # (x_own A)^T, resident

        # PE warm-up tile generated on-chip (iota + cast): no DMA dependency,
        # so the HAM activity window opens while the first operands stream in.
        iti = msc_pool.tile([P, 256], mybir.dt.int32, tag="iti", name="iti")
        nc.gpsimd.iota(iti[:], pattern=[[0, 256]], base=1, channel_multiplier=0)
        warm = msc_pool.tile([P, 256], f32r, tag="warm", name="warm")
        nc.vector.tensor_copy(warm[:], iti[:])
        t3_t = msc_pool.tile([P, KC], f32, tag="t3t", name="t3_t")
        warm_ps = psR_p.tile([1, 256], f32, tag="psR", name="warm_ps")
        for _ in range(16):
            nc.tensor.matmul(warm_ps[:], warm[:, 0:1], warm[:, :])

        # ---- Phase Q: qm^T[e, q] = A^T x_own^T (A resident, phase-scoped) ----
        with tc.tile_pool(name="ap", bufs=1) as a_pool:
            a_t = a_pool.tile([P, KO * D], f32r)
            # first qm iteration needs a[eo=0] plus ALL qc0 columns of xT:
            # interleave those across both queues so they land by ~10.5us,
            # then stream the per-eo A blocks just-in-time on sync.
            nc.sync.dma_start(a_t[:, 0:D], a[:, 0:D])
            for ko in range(KO):
                q = nc.sync if ko % 2 == 0 else nc.gpsimd
                q.dma_start(xt[:, ko, 0:512], xT_r[:, ko, 0:512])
            for eo in range(1, KO):
                nc.sync.dma_start(
                    a_t[:, eo * D : (eo + 1) * D], a[:, eo * D : (eo + 1) * D]
                )
            nc.sync.dma_start(t3_t[:], t3[:, :])
            # own-qc1 columns, then the non-own key columns
            for ko in range(KO):
                nc.gpsimd.dma_start(xt[:, ko, 512:1024], xT_r[:, ko, 512:1024])
            for ko in range(KO):
                nc.gpsimd.dma_start(xt[:, ko, 1024:2048], xT_r[:, ko, 1024:2048])

            for qc in range(2):
                for eo in range(KO):
                    pa = psA_p.tile([P, 512], f32, tag="psA", name="paq")
                    for ko in range(KO):
                        nc.tensor.matmul(
                            pa[:],
                            a_t[:, (eo * KO + ko) * P : (eo * KO + ko + 1) * P],
                            xt[:, ko, qc * 512 : (qc + 1) * 512],
                            start=(ko == 0),
                            stop=(ko == KO - 1),
                        )
                    nc.scalar.copy(qmT[:, eo, qc * 512 : (qc + 1) * 512], pa[:])
                    if qc == 0 and eo < 4:
                        # the early qm groups are HBM-feed-bound; in-order
                        # junk keeps the HAM clock-gate open through stalls
                        for _ in range(6):
                            nc.tensor.matmul(warm_ps[:], warm[:, 0:1], warm[:, :])

        # ---------------- Phase C: attention ----------------
        with (
            tc.tile_pool(name="ep", bufs=1) as e_pool,
            tc.tile_pool(name="vsp", bufs=1) as vs_pool,
            tc.tile_pool(name="osp", bufs=12) as out_pool,
        ):
            xnb_t = vs_pool.tile([P, KC, D], bf16, tag="xnb", name="xnb_t")
            for ko in range(KC):
                nc.gpsimd.dma_start(xnb_t[:, ko, :], xnb_r[:, ko, :])
            wv_sb = [
                vs_pool.tile([P, KO, 512], bf16, tag=f"wv{h}", name=f"wv_sb{h}")
                for h in range(2)
            ]
            for h in range(2):
                for ko in range(KO):
                    nc.gpsimd.dma_start(
                        wv_sb[h][:, ko, :], wvT_r[:, ko, h * 512 : (h + 1) * 512]
                    )
            bvb_t = msc_pool.tile([P, D], f32, tag="bvb", name="bvb_t")
            nc.gpsimd.dma_start(bvb_t[:], bvb[:, :])
            pxt_sb = vs_pool.tile([P, KO, 512], bf16, tag="pxt", name="pxt_sb")

            for qc in range(2):
                E = e_pool.tile([P, KC, 512], bf16, tag="E", name="E")
                racc = msc_pool.tile([P, 512], f32r, tag="racc", name="racc")
                for kc in range(KC):
                    pa = psA_p.tile([P, 512], f32, tag="psA", name="pas")
                    for eo in range(KO):
                        nc.tensor.matmul(
                            pa[:],
                            xt[:, eo, kc * P : (kc + 1) * P],
                            qmT[:, eo, qc * 512 : (qc + 1) * 512],
                            start=(eo == 0),
                            stop=(eo == KO - 1),
                        )
                    nc.scalar.activation(
                        E[:, kc, :], pa[:], AF.Exp, scale=inv_sqrt_dk,
                        bias=t3_t[:, kc : kc + 1],
                    )
                    if kc == 0:
                        nc.vector.tensor_copy(racc[:], E[:, 0, :])
                    else:
                        nc.vector.tensor_add(racc[:], racc[:], E[:, kc, :])
                # partition-reduce rowsum with one ones-matmul, then
                # [1,512] -> per-partition recips [128,4] via DRAM bounce
                pr = psR_p.tile([1, 512], f32, tag="psR", name="pr")
                nc.tensor.matmul(pr[:], warm[:, 0:1], racc[:])
                rsum_row = msc_pool.tile([1, 512], f32, tag="rsr", name="rsum_row")
                nc.scalar.copy(rsum_row[:], pr[:])
                rs_dram = dram_p.tile([1, 512], f32, tag="rsd", name="rs_dram")
                nc.sync.dma_start(rs_dram[:, :], rsum_row[:, :])
                rsum_t = msc_pool.tile([P, 4], f32, tag="rst", name="rsum_t")
                nc.sync.dma_start(
                    rsum_t[:, :], rs_dram[0, :].rearrange("(qs p) -> p qs", p=P)
                )
                recip = msc_pool.tile([P, 4], f32, tag="recip", name="recip")
                nc.vector.reciprocal(recip[:], rsum_t[:])

                # PX^T[d, q] = sum_k x[k, d] E[k, q]: fully SBUF-fed (bf16).
                # bank order: outMM consumes psB/psC first, so evac them first
                pxt_ps = [
                    psB_p.tile([P, 512], f32, tag="psB", name="px0"),
                    psC_p.tile([P, 512], f32, tag="psC", name="px1"),
                    psB_p.tile([P, 512], f32, tag="psB", name="px2"),
                    psC_p.tile([P, 512], f32, tag="psC", name="px3"),
                    psA_p.tile([P, 512], f32, tag="psA", name="px4"),
                    psA_p.tile([P, 512], f32, tag="psA", name="px5"),
                    psA_p.tile([P, 512], f32, tag="psA", name="px6"),
                    psR_p.tile([P, 512], f32, tag="psR", name="px7"),
                ]
                for ko in range(KC):
                    for dc in range(KO):
                        nc.tensor.matmul(
                            pxt_ps[dc][:],
                            xnb_t[:, ko, dc * P : (dc + 1) * P],
                            E[:, ko, :],
                            start=(ko == 0),
                            stop=(ko == KC - 1),
                        )
                for dc in range(KO):
                    nc.vector.tensor_copy(pxt_sb[:, dc, :], pxt_ps[dc][:])
                # out[q, e] = PX^T.T @ Wv^T, scaled by 1/rowsum, + bv
                for qs in range(4):
                    pb = psB_p.tile([P, 512], f32, tag="psB", name="avb")
                    pc = psC_p.tile([P, 512], f32, tag="psC", name="avc")
                    for ko in range(KO):
                        lh = pxt_sb[:, ko, qs * P : (qs + 1) * P]
                        nc.tensor.matmul(
                            pb[:], lh, wv_sb[0][:, ko, :],
                            start=(ko == 0), stop=(ko == KO - 1),
                        )
                        nc.tensor.matmul(
                            pc[:], lh, wv_sb[1][:, ko, :],
                            start=(ko == 0), stop=(ko == KO - 1),
                        )
                    row0 = qc * 512 + qs * P
                    for quarter in range(4):
                        ps = pb if quarter < 2 else pc
                        c0 = quarter * 256
                        o = out_pool.tile([P, 256], f32, tag="ost", name="ost")
                        nc.scalar.activation(
                            o[:], ps[:, (quarter % 2) * 256 : (quarter % 2) * 256 + 256],
                            AF.Identity, scale=recip[:, qs : qs + 1],
                        )
                        nc.vector.tensor_add(o[:], o[:], bvb_t[:, c0 : c0 + 256])
                        nc.sync.dma_start(out[row0 : row0 + P, c0 : c0 + 256], o[:])

    nc.finalize()
    return nc


def make_in_maps(x, Wq, bq, Wk, bk, Wv, bv):
    """Build the 8 per-core input maps from full inputs."""
    import ml_dtypes

    bf16 = ml_dtypes.bfloat16
    x = np.asarray(x, dtype=np.float32)
    # weight-only folding: scores = x (Wq^T Wk) x^T + per-key bias x.(Wk^T bq)
    # (+ per-query consts, softmax-invariant, dropped)
    A = (np.asarray(Wq, np.float64).T @ np.asarray(Wk, np.float64)).astype(np.float32)
    # per-partition-contiguous layout: a_pre[p, eo*1024 + ko*128 + e]
    #   = A[ko*128 + p, eo*128 + e] -> each partition reads 4 KiB bursts
    a_pre = np.ascontiguousarray(
        A.reshape(KO, P, KO, P).transpose(1, 2, 0, 3).reshape(P, KO * D)
    )
    wvT = np.ascontiguousarray(np.asarray(Wv, np.float32).T.astype(bf16))
    w3 = (np.asarray(Wk, np.float64).T @ np.asarray(bq, np.float64)).astype(np.float32)
    inv_sqrt_dk = 1.0 / math.sqrt(D // 16)
    bvb = np.ascontiguousarray(np.broadcast_to(np.asarray(bv, np.float32), (P, D)))

    in_maps = []
    for c in range(N_CORES):
        b, h = c // 2, c % 2
        # key-permute so the core's own query half sits at columns/rows 0..1023
        perm = (
            np.arange(S)
            if h == 0
            else np.concatenate([np.arange(SQ, S), np.arange(0, SQ)])
        )
        xp = x[b][perm]  # [S, D], rows permuted
        t3v = (xp @ w3) * inv_sqrt_dk  # [S]
        in_maps.append(
            {
                "xT": np.ascontiguousarray(xp.T),
                "a": a_pre,
                "xnb": np.ascontiguousarray(xp.astype(bf16)),
                "wvT": wvT,
                "t3": np.ascontiguousarray(t3v.reshape(KC, P).T),
                "bvb": bvb,
            }
        )
    return in_maps


_NC_CACHE = None


def get_nc():
    global _NC_CACHE
    if _NC_CACHE is None:
        _NC_CACHE = build_bass()
    return _NC_CACHE


def kernel(x, Wq, bq, Wk, bk, Wv, bv, **run_kwargs):
    from concourse.bass_utils import run_bass_kernel_spmd

    nc = get_nc()
    in_maps = make_in_maps(x, Wq, bq, Wk, bk, Wv, bv)
    res = run_bass_kernel_spmd(
        nc, in_maps, core_ids=list(range(N_CORES)), **run_kwargs
    )
    out = np.empty((B, S, D), dtype=np.float32)
    for c in range(N_CORES):
        b, h = c // 2, c % 2
        out[b, h * SQ : (h + 1) * SQ, :] = res.results[c]["out"]
    if run_kwargs.get("trace"):
        kernel.last_results = res
    return out
